# revision 33
# baseline (speedup 1.0000x reference)
"""LinearCondensed kernel for Trainium2 (8 NeuronCores).

Reference computation:
    out[b, o] = sum_f input[b, indx_seqs[o, f]] * weight[o, f] + bias[o]
    input: (512, 4096) f32, weight: (4096, 128) f32, bias: (4096,) f32,
    indx_seqs: (4096, 128) int in [0, 4096).

Strategy:
    The gather-modulated contraction is recast as a dense matmul with a
    scattered weight matrix:
        W_dense[o, j] = sum_{f: indx[o,f]=j} weight[o, f]
        out = input @ W_dense^T + bias
    out_features are sharded across the 8 cores (512 outputs per core,
    input replicated). Per core, per 128-wide j-chunk c:
        lhsT = input^T chunk [128 j, 128 b]   (stationary)
        rhs  = W_dense^T chunk [128 j, 512 o] (moving)
        psum[b-block] += lhsT.T @ rhs          (32 chunks accumulated)
    followed by a DVE bias add and a DMA of the naturally-laid-out result.

    Variant "fp16_dense" (default): host-scatters W_dense^T, ships it and
    input^T as fp16 (~9.3 MB DMA per core), runs fp16 matmuls with the bias
    folded in as a K=1 accumulation step, PE warm-up while the first DMAs
    are in flight, uniform 256 KB chunked transfers so the PE chases the
    DMA stream.  Rel err ~3e-4 (fp16 operand rounding, fp32 accumulate).
    Variant "fp16_scatter": ships the sparse (o, w) lists per j-row and
    builds W_dense^T on-device with gpsimd local_scatter (~6.3 MB DMA,
    but the Pool-engine scatter chain is slower than simply streaming the
    dense fp16 chunks).
    Variant "fp32r_dense": W_dense^T densely in fp32r (~17.3 MB DMA,
    DMA-bound; rel err ~1.5e-4 — fallback if tighter precision needed).
"""

import os
import numpy as np

BATCH = 512
IN_WIDTH = 4096
OUT_FEATURES = 4096
FAN_IN = 128
N_CORES = 8
O_PER_CORE = OUT_FEATURES // N_CORES  # 512
N_JCHUNK = IN_WIDTH // 128  # 32
N_BBLK = BATCH // 128  # 4
DMA_GROUP = 2  # j-chunks per input DMA transfer
L_SC = 64  # padded scatter-list length per j-row (expected ~16, Poisson)

VARIANT = os.environ.get("LC_VARIANT", "fp16_dense")

_NC = {}


def _build_nc_fp32r(repeat=1):
    import concourse.bass as bass
    import concourse.tile as tile
    from concourse import bacc, mybir

    f32 = mybir.dt.float32
    f32r = mybir.dt.float32r

    nc = bacc.Bacc("TRN2", target_bir_lowering=False, debug=False)
    inputT = nc.dram_tensor("inputT", (IN_WIDTH, BATCH), f32r, kind="ExternalInput").ap()
    wT = nc.dram_tensor("wT", (IN_WIDTH, O_PER_CORE), f32r, kind="ExternalInput").ap()
    bias_rep = nc.dram_tensor("bias_rep", (128, O_PER_CORE), f32, kind="ExternalInput").ap()
    out = nc.dram_tensor("out", (BATCH, O_PER_CORE), f32, kind="ExternalOutput").ap()

    n_groups = N_JCHUNK // DMA_GROUP

    with tile.TileContext(nc) as tc:
        with (
            tc.tile_pool(name="xp", bufs=1) as xp,
            tc.tile_pool(name="wp", bufs=1) as wp,
            tc.tile_pool(name="op", bufs=1) as op,
            tc.tile_pool(name="ps", bufs=1, space=bass.MemorySpace.PSUM) as psp,
        ):
            bias_t = op.tile([128, O_PER_CORE], f32, tag="bias", name="bias_t")
            nc.sync.dma_start(bias_t[:], bias_rep[:])

            for rep in range(repeat):
                xtiles = []
                wtiles = []
                for g in range(n_groups):
                    xt = xp.tile(
                        [128, DMA_GROUP, BATCH], f32r, tag=f"x{g}", name=f"x{g}_{rep}"
                    )
                    xsrc = inputT[
                        g * DMA_GROUP * 128 : (g + 1) * DMA_GROUP * 128, :
                    ].rearrange("(c p) b -> p c b", p=128)
                    nc.sync.dma_start(xt[:], xsrc)
                    xtiles.append(xt)

                    wt = wp.tile(
                        [128, DMA_GROUP, O_PER_CORE],
                        f32r,
                        tag=f"w{g}",
                        name=f"w{g}_{rep}",
                    )
                    wsrc = wT[
                        g * DMA_GROUP * 128 : (g + 1) * DMA_GROUP * 128, :
                    ].rearrange("(c p) o -> p c o", p=128)
                    nc.sync.dma_start(wt[:], wsrc)
                    wtiles.append(wt)

                psum = [
                    psp.tile(
                        [128, O_PER_CORE], f32, tag=f"ps{bb}", name=f"ps{bb}_{rep}"
                    )
                    for bb in range(N_BBLK)
                ]

                for g in range(n_groups):
                    for cl in range(DMA_GROUP):
                        c = g * DMA_GROUP + cl
                        for bb in range(N_BBLK):
                            nc.tensor.matmul(
                                psum[bb][:],
                                xtiles[g][:, cl, bass.ts(bb, 128)],
                                wtiles[g][:, cl, :],
                                start=(c == 0),
                                stop=(c == N_JCHUNK - 1),
                            )

                for bb in range(N_BBLK):
                    ot = op.tile(
                        [128, O_PER_CORE], f32, tag=f"ot{bb}", name=f"ot{bb}_{rep}"
                    )
                    nc.vector.tensor_add(ot[:], psum[bb][:], bias_t[:])
                    nc.sync.dma_start(out[bass.ts(bb, 128), :], ot[:])

    nc.compile()
    return nc


DMA_GROUPS = (2,) * 16  # j-chunks per DMA transfer, in order
_SPLIT = 26  # chunk index where per-b-block grouping starts (tail stagger)


def _build_nc_fp16_dense(repeat=1, warmup=2):
    import concourse.bass as bass
    import concourse.tile as tile
    from concourse import bacc, mybir

    f32 = mybir.dt.float32
    f16 = mybir.dt.float16

    assert sum(DMA_GROUPS) == N_JCHUNK

    nc = bacc.Bacc("TRN2", target_bir_lowering=False, debug=False)
    inputT = nc.dram_tensor("inputT", (IN_WIDTH, BATCH), f16, kind="ExternalInput").ap()
    wT = nc.dram_tensor("wT", (IN_WIDTH, O_PER_CORE), f16, kind="ExternalInput").ap()
    bias16 = nc.dram_tensor("bias16", (1, O_PER_CORE), f16, kind="ExternalInput").ap()
    out = nc.dram_tensor("out", (BATCH, O_PER_CORE), f32, kind="ExternalOutput").ap()

    with tile.TileContext(nc) as tc:
        with (
            tc.tile_pool(name="xp", bufs=1) as xp,
            tc.tile_pool(name="wp", bufs=1) as wp,
            tc.tile_pool(name="op", bufs=1) as op,
            tc.tile_pool(name="ps", bufs=1, space=bass.MemorySpace.PSUM) as psp,
        ):
            ones_t = op.tile([1, 128], f16, tag="ones", name="ones_t")
            nc.gpsimd.memset(ones_t[:], 1.0)

            # Small PE warm-up while the first input DMAs are in flight.
            # Tiny N=128 matmuls: the clock-gate ramp needs elapsed time
            # since first PE activity, not work volume.
            if warmup:
                wu = op.tile([128, 128], f16, tag="wu", name="wu")
                nc.gpsimd.memset(wu[:], 0.0)
                pwu = psp.tile([128, 128], f32, tag="pswu", name="pswu")
                for i in range(warmup):
                    nc.tensor.matmul(
                        pwu[:], wu[:], wu[:], start=True, stop=True
                    )

            for rep in range(repeat):
                # chunk c -> (tile index, local offset)
                chunk_loc = []
                xtiles = []
                wtiles = []
                for g, gsz in enumerate(DMA_GROUPS):
                    base = sum(DMA_GROUPS[:g])
                    for cl in range(gsz):
                        chunk_loc.append((g, cl))
                    xt = xp.tile(
                        [128, gsz, BATCH], f16, tag=f"x{g}", name=f"x{g}_{rep}"
                    )
                    xsrc = inputT[
                        base * 128 : (base + gsz) * 128, :
                    ].rearrange("(c p) b -> p c b", p=128)
                    nc.sync.dma_start(xt[:], xsrc)
                    xtiles.append(xt)

                    wt = wp.tile(
                        [128, gsz, O_PER_CORE], f16, tag=f"w{g}", name=f"w{g}_{rep}"
                    )
                    wsrc = wT[
                        base * 128 : (base + gsz) * 128, :
                    ].rearrange("(c p) o -> p c o", p=128)
                    nc.sync.dma_start(wt[:], wsrc)
                    wtiles.append(wt)
                    if g == 1 and rep == 0:
                        # bias load queued after the second chunk pair
                        bias_t = op.tile(
                            [1, O_PER_CORE], f16, tag="bias", name="bias_t"
                        )
                        nc.sync.dma_start(bias_t[:], bias16[:])

                psum = [
                    psp.tile(
                        [128, O_PER_CORE], f32, tag=f"ps{bb}", name=f"ps{bb}_{rep}"
                    )
                    for bb in range(N_BBLK)
                ]

                # chunks 0..split-1: all four b-blocks per chunk;
                # chunks split..31: grouped per b-block so psum[0] finishes
                # (and its copy + out DMA start) while the PE still streams
                # the other blocks' matmuls — hides the output tail.
                split = _SPLIT
                for c in range(split):
                    g, cl = chunk_loc[c]
                    for bb in range(N_BBLK):
                        nc.tensor.matmul(
                            psum[bb][:],
                            xtiles[g][:, cl, bass.ts(bb, 128)],
                            wtiles[g][:, cl, :],
                            start=(c == 0),
                            stop=False,
                        )
                    if c == 0:
                        # fold the bias in as a K=1 accumulation step
                        for bb in range(N_BBLK):
                            nc.tensor.matmul(
                                psum[bb][:],
                                ones_t[:],
                                bias_t[:],
                                start=False,
                                stop=False,
                            )
                for bb in range(N_BBLK):
                    for c in range(split, N_JCHUNK):
                        g, cl = chunk_loc[c]
                        nc.tensor.matmul(
                            psum[bb][:],
                            xtiles[g][:, cl, bass.ts(bb, 128)],
                            wtiles[g][:, cl, :],
                            start=False,
                            stop=(c == N_JCHUNK - 1),
                        )

                # tail: alternating ACT/DVE copies
                for bb in range(N_BBLK):
                    ot = op.tile(
                        [128, O_PER_CORE], f32, tag=f"ot{bb}", name=f"ot{bb}_{rep}"
                    )
                    if bb % 2 == 0:
                        nc.scalar.copy(ot[:], psum[bb][:])
                    else:
                        nc.vector.tensor_copy(ot[:], psum[bb][:])
                    nc.sync.dma_start(out[bass.ts(bb, 128), :], ot[:])

    nc.compile()
    return nc


def _build_nc_fp16(repeat=1):
    import concourse.bass as bass
    import concourse.tile as tile
    from concourse import bacc, mybir, library_config

    f32 = mybir.dt.float32
    f16 = mybir.dt.float16
    i16 = mybir.dt.int16

    nc = bacc.Bacc("TRN2", target_bir_lowering=False, debug=False)
    inputT = nc.dram_tensor("inputT", (IN_WIDTH, BATCH), f16, kind="ExternalInput").ap()
    sc_data = nc.dram_tensor(
        "sc_data", (128, N_JCHUNK, L_SC), f16, kind="ExternalInput"
    ).ap()
    sc_idx = nc.dram_tensor(
        "sc_idx", (128, N_JCHUNK, L_SC), i16, kind="ExternalInput"
    ).ap()
    bias_rep = nc.dram_tensor("bias_rep", (128, O_PER_CORE), f32, kind="ExternalInput").ap()
    out = nc.dram_tensor("out", (BATCH, O_PER_CORE), f32, kind="ExternalOutput").ap()

    n_groups = N_JCHUNK // DMA_GROUP

    with tile.TileContext(nc) as tc:
        with (
            tc.tile_pool(name="xp", bufs=1) as xp,
            tc.tile_pool(name="wp", bufs=1) as wp,
            tc.tile_pool(name="sp", bufs=1) as sp,
            tc.tile_pool(name="op", bufs=1) as op,
            tc.tile_pool(name="ps", bufs=1, space=bass.MemorySpace.PSUM) as psp,
        ):
            nc.gpsimd.load_library(library_config.local_scatter)

            bias_t = op.tile([128, O_PER_CORE], f32, tag="bias", name="bias_t")
            nc.sync.dma_start(bias_t[:], bias_rep[:])

            data_t = sp.tile([128, N_JCHUNK, L_SC], f16, tag="scd", name="data_t")
            nc.sync.dma_start(data_t[:], sc_data[:])
            idx_t = sp.tile([128, N_JCHUNK, L_SC], i16, tag="sci", name="idx_t")
            nc.sync.dma_start(idx_t[:], sc_idx[:])

            for rep in range(repeat):
                xtiles = []
                for g in range(n_groups):
                    xt = xp.tile(
                        [128, DMA_GROUP, BATCH], f16, tag=f"x{g}", name=f"x{g}_{rep}"
                    )
                    xsrc = inputT[
                        g * DMA_GROUP * 128 : (g + 1) * DMA_GROUP * 128, :
                    ].rearrange("(c p) b -> p c b", p=128)
                    nc.sync.dma_start(xt[:], xsrc)
                    xtiles.append(xt)

                wtiles = []
                for c in range(N_JCHUNK):
                    wt = wp.tile(
                        [128, O_PER_CORE], f16, tag=f"w{c}", name=f"w{c}_{rep}"
                    )
                    nc.gpsimd.local_scatter(
                        wt[:],
                        data_t[:, c, :],
                        idx_t[:, c, :],
                        channels=128,
                        num_elems=O_PER_CORE,
                        num_idxs=L_SC,
                    )
                    wtiles.append(wt)

                psum = [
                    psp.tile(
                        [128, O_PER_CORE], f32, tag=f"ps{bb}", name=f"ps{bb}_{rep}"
                    )
                    for bb in range(N_BBLK)
                ]

                for c in range(N_JCHUNK):
                    g, cl = divmod(c, DMA_GROUP)
                    for bb in range(N_BBLK):
                        nc.tensor.matmul(
                            psum[bb][:],
                            xtiles[g][:, cl, bass.ts(bb, 128)],
                            wtiles[c][:],
                            start=(c == 0),
                            stop=(c == N_JCHUNK - 1),
                        )

                for bb in range(N_BBLK):
                    ot = op.tile(
                        [128, O_PER_CORE], f32, tag=f"ot{bb}", name=f"ot{bb}_{rep}"
                    )
                    nc.vector.tensor_add(ot[:], psum[bb][:], bias_t[:])
                    nc.sync.dma_start(out[bass.ts(bb, 128), :], ot[:])

    nc.compile()
    return nc


def _get_nc(repeat=1, variant=None):
    variant = variant or VARIANT
    key = (variant, repeat)
    if key not in _NC:
        if variant == "fp16_scatter":
            _NC[key] = _build_nc_fp16(repeat)
        elif variant == "fp16_dense":
            _NC[key] = _build_nc_fp16_dense(repeat)
        else:
            _NC[key] = _build_nc_fp32r(repeat)
    return _NC[key]


def _scatter_dense(inputs):
    """Host scatter: W_dense^T[j, o] = sum of w[o, f] with idx[o, f] == j."""
    w = np.asarray(inputs["weight"], dtype=np.float32)
    idx = np.asarray(inputs["indx_seqs"])
    wT = np.zeros((IN_WIDTH, OUT_FEATURES), np.float32)
    o_idx = np.repeat(np.arange(OUT_FEATURES, dtype=np.intp), FAN_IN)
    np.add.at(wT, (idx.ravel(), o_idx), w.ravel())
    return wT


def _prepare_in_maps_fp32r(inputs, wT):
    x = np.ascontiguousarray(np.asarray(inputs["input"], dtype=np.float32))
    b = np.asarray(inputs["bias"], dtype=np.float32)
    xT = np.ascontiguousarray(x.T)

    in_maps = []
    for c in range(N_CORES):
        sl = slice(c * O_PER_CORE, (c + 1) * O_PER_CORE)
        in_maps.append(
            {
                "inputT": xT,
                "wT": np.ascontiguousarray(wT[:, sl]),
                "bias_rep": np.ascontiguousarray(
                    np.broadcast_to(b[sl][None, :], (128, O_PER_CORE))
                ),
            }
        )
    return in_maps


def _prepare_in_maps_fp16_dense(inputs, wT):
    x = np.asarray(inputs["input"], dtype=np.float32)
    b = np.asarray(inputs["bias"], dtype=np.float32)
    xT16 = np.ascontiguousarray(x.T.astype(np.float16))
    wT16 = wT.astype(np.float16)

    in_maps = []
    for c in range(N_CORES):
        sl = slice(c * O_PER_CORE, (c + 1) * O_PER_CORE)
        in_maps.append(
            {
                "inputT": xT16,
                "wT": np.ascontiguousarray(wT16[:, sl]),
                "bias16": b[sl].astype(np.float16).reshape(1, -1),
            }
        )
    return in_maps


def _prepare_in_maps_fp16(inputs, wT):
    """Returns in_maps, or None if any scatter list overflows L_SC."""
    x = np.asarray(inputs["input"], dtype=np.float32)
    b = np.asarray(inputs["bias"], dtype=np.float32)
    xT16 = np.ascontiguousarray(x.T.astype(np.float16))

    in_maps = []
    for c in range(N_CORES):
        sl = slice(c * O_PER_CORE, (c + 1) * O_PER_CORE)
        wTc = wT[:, sl]
        jj, oo = np.nonzero(wTc)
        vals = wTc[jj, oo].astype(np.float16)
        starts = np.searchsorted(jj, np.arange(IN_WIDTH))
        pos = np.arange(len(jj)) - starts[jj]
        if len(pos) and pos.max() >= L_SC:
            return None
        blk = jj >> 7
        p = jj & 127
        data = np.zeros((128, N_JCHUNK, L_SC), np.float16)
        idxs = np.full((128, N_JCHUNK, L_SC), -1, np.int16)
        data[p, blk, pos] = vals
        idxs[p, blk, pos] = oo.astype(np.int16)
        in_maps.append(
            {
                "inputT": xT16,
                "sc_data": data,
                "sc_idx": idxs,
                "bias_rep": np.ascontiguousarray(
                    np.broadcast_to(b[sl][None, :], (128, O_PER_CORE))
                ),
            }
        )
    return in_maps


def run(inputs, trace=False):
    """Run the kernel; returns (output, BassKernelResults)."""
    from concourse.bass_utils import run_bass_kernel_spmd

    wT = _scatter_dense(inputs)
    variant = VARIANT
    in_maps = None
    if variant == "fp16_scatter":
        in_maps = _prepare_in_maps_fp16(inputs, wT)
        if in_maps is None:
            variant = "fp32r_dense"
    elif variant == "fp16_dense":
        in_maps = _prepare_in_maps_fp16_dense(inputs, wT)
    if in_maps is None:
        in_maps = _prepare_in_maps_fp32r(inputs, wT)

    nc = _get_nc(variant=variant)
    res = run_bass_kernel_spmd(
        nc, in_maps, core_ids=list(range(N_CORES)), trace=trace
    )
    out = np.concatenate(
        [res.results[c]["out"] for c in range(N_CORES)], axis=1
    )
    return out, res


def kernel(**inputs) -> np.ndarray:
    out, _ = run(inputs, trace=False)
    return out
